# revision 28
# baseline (speedup 1.0000x reference)
"""CARAFE (content-aware upsampling) Trainium2 kernel.

Full inputs -> shard over 8 NeuronCores (batch x image-half) -> bass/Tile
kernel per core -> gather full output.

Reference semantics:
  comp = conv1x1(x, w_comp) + b_comp                    [n,64,64,64]
  mask = conv3x3(comp, w_enc, pad=1) + b_enc            [n,100,64,64]
  m    = softmax over 25 of pixel_shuffle(mask, 2)      [n,25,128,128]
  out[n,c,i,j] = sum_k m[n,k,i,j] * xpad[n,c,i//2+p, j//2+q],  k=5p+q

Band construction (v3): per h-pair block and low-res row r, mask values
live in a DRAM "band" image of 68 rows x 1280 cols:
  row = jl + q   (rows 0,1,66,67 are pad; valid w' = row-2)
  col = 20*jl + 10*a + 2*p + b
Nonzero cells of each row form one contiguous 100-el run, so the
SBUF->DRAM scatter is 1 flat shear DMA per (block, r).  Bands are read
back as [64 w', 1280] tiles and contracted against per-row xT tiles on
the PE array.  4-slot ring, zeroed once; slot cells are overwritten in
place each block so no re-zeroing is needed.
"""
import numpy as np
import sys
from contextlib import ExitStack

sys.path.insert(0, "/opt/trn_rl_repo")

# ---------------- problem constants (hardcoded per spec) ----------------
N_B, C, H, W = 4, 256, 64, 64
CC = 64            # compressed channels
K5 = 5             # carafe kernel
S = 2              # scale
CM = K5 * K5 * S * S   # 100 mask channels
NCORES = 8
RH = H // 2        # 32 low-res rows per core
SLAB = RH + 4      # 36 x-rows per core (h0-2 .. h0+33)
NBLK = RH // 2     # 16 h-pair blocks
PIXC = RH * W      # 2048 low-res pixels per core
HO, WO = 2 * RH, 2 * W   # 64 x 128 output shard

NRING = 4          # band ring slots
BW = 20 * W        # 1280 band cols: (jl,a,p,b)
BROWS = W + 4      # 68 band rows: jl + q, 2 pad rows each side

_MM_DT = "float32r"   # comp/mask conv matmul dtype
_BD_DT = "bfloat16"   # carafe path: xT, softmax mask, band, output


def _build_program():
    import concourse.bass as bass
    import concourse.tile as tile
    from concourse import bacc, mybir
    from concourse.ap import AP
    AF = mybir.ActivationFunctionType

    def pstep(t):
        return t[:].ap[0][0]

    f32 = mybir.dt.float32
    mmdt = getattr(mybir.dt, _MM_DT)
    bddt = getattr(mybir.dt, _BD_DT)

    nc = bacc.Bacc("TRN2", target_bir_lowering=False, debug=False,
                   num_devices=NCORES)

    # ---------------- DRAM parameters ----------------
    xs = nc.dram_tensor("xs", [C + 1, SLAB, W], mmdt, kind="ExternalInput")
    xsT = nc.dram_tensor("xsT", [SLAB * W, C], bddt, kind="ExternalInput")
    bndz = nc.dram_tensor("bndz", [NRING, 2, BROWS, BW], bddt)
    wcT = nc.dram_tensor("wcT", [C, CC], mmdt, kind="ExternalInput")
    bcr = nc.dram_tensor("bcr", [1, CC], mmdt, kind="ExternalInput")
    weT = nc.dram_tensor("weT", [9, CC, CM], mmdt, kind="ExternalInput")
    be = nc.dram_tensor("be", [CM, 1], f32, kind="ExternalInput")
    sel01 = nc.dram_tensor("sel01", [CM, 4], mmdt, kind="ExternalInput")
    ident = nc.dram_tensor("ident", [128, 128], mmdt, kind="ExternalInput")
    out = nc.dram_tensor("out", [C, HO, WO], bddt, kind="ExternalOutput")

    COMP_W = W + 2      # 66: comp cols with 1 zero col each side

    with tile.TileContext(nc) as tc:
        with ExitStack() as ctx:
            cpool = ctx.enter_context(tc.tile_pool(name="const", bufs=1))
            xpool = ctx.enter_context(tc.tile_pool(name="xdata", bufs=1))
            work = ctx.enter_context(tc.tile_pool(name="work", bufs=3))
            opool = ctx.enter_context(tc.tile_pool(name="oevac", bufs=4))
            pers = ctx.enter_context(tc.tile_pool(name="pers", bufs=1))
            bpool = ctx.enter_context(tc.tile_pool(name="bndsb", bufs=3))
            ps_comp_p = ctx.enter_context(tc.tile_pool(name="pscomp", bufs=1, space="PSUM"))
            ps_mask_p = ctx.enter_context(tc.tile_pool(name="psmask", bufs=1, space="PSUM"))
            ps_z_p = ctx.enter_context(tc.tile_pool(name="psz", bufs=1, space="PSUM"))
            ps_t_p = ctx.enter_context(tc.tile_pool(name="pst", bufs=1, space="PSUM"))
            ps_o_p = ctx.enter_context(tc.tile_pool(name="pso", bufs=2, space="PSUM"))

            # ---------------- load constants (scalar queue) ----------------
            t_wc = [cpool.tile([128, CC], mmdt, tag=f"wc{k}", name=f"wc{k}") for k in range(2)]
            for k in range(2):
                nc.scalar.dma_start(t_wc[k][:], wcT.ap()[128 * k:128 * (k + 1), :])
            t_bcr = cpool.tile([1, CC], mmdt, tag="bcr", name="bcr")
            nc.scalar.dma_start(t_bcr[:], bcr.ap())
            # encoder weights duplicated into partitions [64,128) so odd taps
            # can run in PE row-group 2-3 (row tiling)
            t_we = cpool.tile([128, 9 * CM], mmdt, tag="we", name="we")
            src_we = AP(weT.ap().tensor, 0, [[CM, CC], [CC * CM, 9], [1, CM]])
            nc.scalar.dma_start(t_we[0:CC, :], src_we)
            nc.scalar.dma_start(t_we[CC:2 * CC, :], src_we)
            t_be = cpool.tile([CM, 1], f32, tag="be", name="be")
            nc.sync.dma_start(t_be[:], be.ap())
            t_sel = cpool.tile([CM, 4], mmdt, tag="sel", name="sel")
            nc.sync.dma_start(t_sel[:], sel01.ap())
            t_id = cpool.tile([128, 128], mmdt, tag="id", name="id")
            nc.sync.dma_start(t_id[:], ident.ap())

            # ---------------- load x (scalar queue, in slab-row chunks so
            # the comp conv can start before the whole load finishes) ------
            t_x = [xpool.tile([128, SLAB * W], mmdt, tag=f"x{k}", name=f"x{k}") for k in range(2)]
            cuts = [0, 7 * W, 12 * W, 17 * W, 23 * W, 29 * W, SLAB * W]
            for h in range(len(cuts) - 1):
                for k in range(2):
                    nc.scalar.dma_start(
                        t_x[k][:, cuts[h]:cuts[h + 1]],
                        xs.ap()[128 * k:128 * (k + 1), :, :]
                        .rearrange("p r w -> p (r w)")[:, cuts[h]:cuts[h + 1]])
            t_ones = xpool.tile([1, SLAB * W], mmdt, tag="ones", name="ones")
            nc.scalar.dma_start(t_ones[:], xs.ap()[C:C + 1, :, :])

            # xT: 35 overlapping row-pair tiles [128=(2 rows x 64 w), 256 c]
            # bf16.  Row s sits at partitions [0,64) of tile s AND [64,128)
            # of tile s-1, so both PE row-groups can be fed for row tiling.
            # All on the sync queue, which is otherwise idle during startup.
            t_xT = [xpool.tile([128, C], bddt, tag=f"xT{t}", name=f"xT{t}")
                    for t in range(SLAB - 1)]
            for t in range(SLAB - 1):
                nc.sync.dma_start(t_xT[t][:], xsT.ap()[W * t:W * t + 128, :])

            # ------------- zero the band ring once (gpsimd sw queue) -------
            t_zero = pers.tile([BROWS, BW], bddt, tag="zero", name="zero")
            nc.vector.memset(t_zero[:], 0.0)
            for s in range(NRING):
                for r in range(2):
                    nc.gpsimd.dma_start(bndz.ap()[s, r, :, :], t_zero[:])

            # ---------------- comp = 1x1 conv + bias (rows 1..34 of slab) ----
            # comp stored [128, 34, 66]: partitions [64,128) duplicate
            # [0,64) so odd mask-conv taps can use PE row-group 2-3.
            # Zero cols 0 and 65.
            t_comp = pers.tile([128, (RH + 2) * COMP_W], mmdt, tag="comp", name="comp")
            compv = t_comp[:].rearrange("p (r w) -> p r w", w=COMP_W)
            nc.vector.memset(compv[:, :, 0:1].bitcast(f32), 0.0)
            nc.vector.memset(compv[:, :, COMP_W - 1:COMP_W].bitcast(f32), 0.0)

            NPIX_C = (RH + 2) * W  # 2176 pixels (rows 1..34 of slab)
            ctile = 512
            nct = (NPIX_C + ctile - 1) // ctile
            for nt in range(nct):
                p0 = nt * ctile
                n = min(ctile, NPIX_C - p0)
                ps = ps_comp_p.tile([CC, ctile], f32, tag="ps_comp", name="ps_comp")
                for k in range(2):
                    rhs = AP(t_x[k][:].tensor, W + p0, [[pstep(t_x[k]), 128], [1, n]])
                    nc.tensor.matmul(ps[:, :n], t_wc[k][:], rhs,
                                     start=(k == 0), stop=False)
                rhs1 = AP(t_ones[:].tensor, W + p0, [[pstep(t_ones), 1], [1, n]])
                nc.tensor.matmul(ps[:, :n], t_bcr[:], rhs1, start=False, stop=True)
                # evacuate into comp[., rows, 1:65], both partition halves
                r0 = p0 // W
                nr = n // W
                src_ps = ps[:, :n].rearrange("p (r w) -> p r w", w=W)
                nc.scalar.activation(compv[0:CC, r0:r0 + nr, 1:1 + W], src_ps,
                                     func=AF.Copy)
                nc.scalar.activation(compv[CC:2 * CC, r0:r0 + nr, 1:1 + W], src_ps,
                                     func=AF.Copy)

            # ---------------- mask conv 3x3 -> exp -> Z ----------------
            # emask [104, RH*W]: rows 0..99 exp(mask), rows 100..103 Z per ab
            t_em = pers.tile([CM + 4, PIXC], mmdt, tag="emask", name="emask")
            emv = t_em[:].rearrange("p (r w) -> p r w", w=W)
            mtile = 512
            for nt in range(PIXC // mtile):
                mr0 = nt * mtile // W   # 8 mask rows per tile
                ps = ps_mask_p.tile([CM, mtile], f32, tag="ps_mask", name="ps_mask")
                first = True
                for dy in range(3):
                    for dx in range(3):
                        tap = dy * 3 + dx
                        g = 0   # HW rejects accum chains across row-groups
                        rhs = compv[g:g + CC, mr0 + dy:mr0 + dy + 8, dx:dx + W]
                        nc.tensor.matmul(ps[:],
                                         t_we[g:g + CC, tap * CM:(tap + 1) * CM],
                                         rhs, start=first,
                                         stop=(tap == 8))
                        first = False
                # exp(mask + be) -> emask rows 0..99
                dst = emv[0:CM, mr0:mr0 + 8, :]
                nc.scalar.activation(dst, ps[:].rearrange("p (r w) -> p r w", w=W),
                                     func=AF.Exp, bias=t_be[:])
                # Z = sel01.T @ emask chunk -> rows 100..104 (via staging DMA:
                # engine writes must start at partition 0/32/64/96)
                psz = ps_z_p.tile([4, mtile], f32, tag="ps_z", name="ps_z")
                nc.tensor.matmul(psz[:], t_sel[:], t_em[0:CM, nt * mtile:(nt + 1) * mtile],
                                 start=True, stop=True)
                t_zs = work.tile([4, mtile], mmdt, tag="zstage", name="zstage")
                nc.scalar.activation(t_zs[:], psz[:], func=AF.Copy)
                nc.gpsimd.dma_start(emv[CM:CM + 4, mr0:mr0 + 8, :],
                                    t_zs[:].rearrange("p (r w) -> p r w", w=W))

            # ---------------- per h-pair block ----------------
            for t in range(NBLK):
                slot = t % NRING
                # transpose emask[:, 2t:2t+2, :] -> [128 pix, 104]
                psT = ps_t_p.tile([128, CM + 4], mmdt, tag="ps_T", name="ps_T")
                src = emv[:, 2 * t:2 * t + 2, :].rearrange("p a b -> p (a b)")
                nc.tensor.transpose(psT[:], src, t_id[0:CM + 4, 0:CM + 4])
                t_emT = work.tile([128, CM + 4], f32, tag="emT", name="emT")
                nc.scalar.activation(t_emT[:], psT[:], func=AF.Copy)

                # reciprocal of Z (4 cols)
                t_rz = work.tile([128, 4], f32, tag="rz", name="rz")
                nc.vector.reciprocal(t_rz[:], t_emT[:, CM:CM + 4])

                # msoftT = emT[:, 0:100] * rz[ch%4]  (softmax normalize)
                t_ms = work.tile([128, CM], bddt, tag="msoft", name="msoft")
                in0 = t_emT[:, 0:CM].rearrange("p (k ab) -> p k ab", ab=4)
                in1 = AP(t_rz[:].tensor, 0, [[pstep(t_rz), 128], [0, 25], [1, 4]])
                out_ms = t_ms[:].rearrange("p (k ab) -> p k ab", ab=4)
                nc.vector.tensor_mul(out_ms, in0, in1)

                # R'' permutation: col (q, a, p, b) <- ms ch 20p+4q+2a+b
                t_rp = work.tile([128, CM], bddt, tag="rpp", name="rpp")
                rps = pstep(t_rp)
                msp = pstep(t_ms)
                for a in range(2):
                    dstp = AP(t_rp[:].tensor, 10 * a,
                              [[rps, 128], [20, 5], [2, 5], [1, 2]])
                    srcp = AP(t_ms[:].tensor, 2 * a,
                              [[msp, 128], [4, 5], [20, 5], [1, 2]])
                    nc.vector.tensor_copy(dstp, srcp)

                # band scatter: 1 DMA per r, split across the two HW queues.
                # row = jl+q, col = 20jl+10a+2p+b
                with nc.allow_non_contiguous_dma(reason="banded mask scatter"):
                    for r in range(2):
                        dstb = AP(bndz.ap().tensor,
                                  (slot * 2 + r) * BROWS * BW,
                                  [[BW + 20, W], [BW, K5], [1, 20]])
                        srcb = AP(t_rp[:].tensor, (r * W) * rps,
                                  [[rps, W], [20, K5], [1, 20]])
                        eng = nc.sync if r == 0 else nc.scalar
                        eng.dma_start(dstb, srcb)

                # band readback: rows [2,66) of each r section; r=1 lands in
                # partitions [64,128) so its matmuls run in PE row-group 2-3
                # (row tiling) concurrently with r=0 in row-group 0-1.
                bnd = bpool.tile([128, BW], bddt, tag="bnd", name="bnd")
                bps = pstep(bnd)
                for r in range(2):
                    srcr = AP(bndz.ap().tensor,
                              (slot * 2 + r) * BROWS * BW + 2 * BW,
                              [[BW, W], [1, BW]])
                    eng = nc.sync if r == 0 else nc.scalar
                    eng.dma_start(bnd[64 * r:64 * (r + 1), :], srcr)

                # CARAFE: psum[c, (a,jl,b)] += xT_row.T @ band_p
                for ct in range(2):
                    pso = [ps_o_p.tile([128, 256], f32, tag=f"ps_o{r}",
                                       name=f"ps_o{r}") for r in range(2)]
                    for p in range(K5):
                        for r in range(2):
                            rhs = AP(bnd[:].tensor, 64 * r * bps + 2 * p,
                                     [[bps, W], [10, 2], [20, W], [1, 2]])
                            nc.tensor.matmul(
                                pso[r][:],
                                t_xT[2 * t + p][64 * r:64 * (r + 1),
                                                128 * ct:128 * (ct + 1)],
                                rhs, start=(p == 0), stop=(p == K5 - 1))
                    t_o = opool.tile([128, 512], bddt, tag=f"osb{ct}", name=f"osb{ct}")
                    for r in range(2):
                        nc.vector.tensor_copy(t_o[:, 256 * r:256 * (r + 1)], pso[r][:])
                    # DMA to out[c, hr, j]: cols (r,a,jl,b) = 4 contiguous hr rows
                    dsto = AP(out.ap().tensor,
                              ct * 128 * HO * WO + 4 * t * WO,
                              [[HO * WO, 128], [1, 512]])
                    eng = nc.scalar if ct == 0 else nc.sync
                    eng.dma_start(dsto, t_o[:])

    nc.compile()
    return nc


_CACHE = {}


def _get_program():
    if "nc" not in _CACHE:
        _CACHE["nc"] = _build_program()
    return _CACHE["nc"]


def host_prep(x, w_comp, b_comp, w_enc, b_enc):
    """Build per-core input maps."""
    import ml_dtypes
    bf16 = ml_dtypes.bfloat16
    x = np.asarray(x, dtype=np.float32)
    wcT = np.ascontiguousarray(np.asarray(w_comp, np.float32).reshape(CC, C).T)
    bcr = np.asarray(b_comp, np.float32).reshape(1, CC)
    # weT[tap, cin, cout]
    weT = np.ascontiguousarray(
        np.asarray(w_enc, np.float32).reshape(CM, CC, 9).transpose(2, 1, 0))
    be = np.asarray(b_enc, np.float32).reshape(CM, 1)
    sel = np.zeros((CM, 4), np.float32)
    sel[np.arange(CM), np.arange(CM) % 4] = 1.0
    ident = np.eye(128, dtype=np.float32)

    in_maps = []
    for core in range(NCORES):
        n, half = core // 2, core % 2
        h0 = RH * half
        slab = np.zeros((C + 1, SLAB, W), np.float32)
        r_lo, r_hi = h0 - 2, h0 + SLAB - 2       # x rows [r_lo, r_hi)
        v_lo, v_hi = max(0, r_lo), min(H, r_hi)
        slab[:C, v_lo - r_lo:v_hi - r_lo, :] = x[n, :, v_lo:v_hi, :]
        slab[C, v_lo - r_lo:v_hi - r_lo, :] = 1.0
        xsT = np.ascontiguousarray(
            slab[:C].reshape(C, SLAB * W).T).astype(bf16)
        in_maps.append({"xs": slab, "xsT": xsT, "wcT": wcT, "bcr": bcr,
                        "weT": weT, "be": be, "sel01": sel, "ident": ident})
    return in_maps


def host_gather(results):
    out = np.empty((N_B, C, S * H, S * W), np.float32)
    for core in range(NCORES):
        n, half = core // 2, core % 2
        out[n, :, HO * half:HO * (half + 1), :] = \
            results[core]["out"].astype(np.float32)
    return out


def kernel(x, w_comp, b_comp, w_enc, b_enc):
    from concourse.bass_utils import run_bass_kernel_spmd
    nc = _get_program()
    in_maps = host_prep(x, w_comp, b_comp, w_enc, b_enc)
    res = run_bass_kernel_spmd(nc, in_maps, list(range(NCORES)))
    return host_gather(res.results)


# revision 31
# speedup vs baseline: 1.1605x; 1.1605x over previous
"""CARAFE (content-aware upsampling) Trainium2 kernel.

Full inputs -> shard over 8 NeuronCores (batch x image-half) -> bass/Tile
kernel per core -> gather full output.

Reference semantics:
  comp = conv1x1(x, w_comp) + b_comp                    [n,64,64,64]
  mask = conv3x3(comp, w_enc, pad=1) + b_enc            [n,100,64,64]
  m    = softmax over 25 of pixel_shuffle(mask, 2)      [n,25,128,128]
  out[n,c,i,j] = sum_k m[n,k,i,j] * xpad[n,c,i//2+p, j//2+q],  k=5p+q

Band construction (v3): per h-pair block and low-res row r, mask values
live in a DRAM "band" image of 68 rows x 1280 cols:
  row = jl + q   (rows 0,1,66,67 are pad; valid w' = row-2)
  col = 20*jl + 10*a + 2*p + b
Nonzero cells of each row form one contiguous 100-el run, so the
SBUF->DRAM scatter is 1 flat shear DMA per (block, r).  Bands are read
back as [64 w', 1280] tiles and contracted against per-row xT tiles on
the PE array.  4-slot ring, zeroed once; slot cells are overwritten in
place each block so no re-zeroing is needed.
"""
import numpy as np
import sys
from contextlib import ExitStack

sys.path.insert(0, "/opt/trn_rl_repo")

# ---------------- problem constants (hardcoded per spec) ----------------
N_B, C, H, W = 4, 256, 64, 64
CC = 64            # compressed channels
K5 = 5             # carafe kernel
S = 2              # scale
CM = K5 * K5 * S * S   # 100 mask channels
NCORES = 8
RH = H // 2        # 32 low-res rows per core
SLAB = RH + 4      # 36 x-rows per core (h0-2 .. h0+33)
NBLK = RH // 2     # 16 h-pair blocks
PIXC = RH * W      # 2048 low-res pixels per core
HO, WO = 2 * RH, 2 * W   # 64 x 128 output shard

NRING = 4          # band ring slots
BW = 20 * W        # 1280 band cols: (jl,a,p,b)
BROWS = W + 4      # 68 band rows: jl + q, 2 pad rows each side

_MM_DT = "float32r"   # comp/mask conv matmul dtype
_BD_DT = "bfloat16"   # carafe path: xT, softmax mask, band, output


def _build_program():
    import concourse.bass as bass
    import concourse.tile as tile
    from concourse import bacc, mybir
    from concourse.ap import AP
    AF = mybir.ActivationFunctionType

    def pstep(t):
        return t[:].ap[0][0]

    f32 = mybir.dt.float32
    mmdt = getattr(mybir.dt, _MM_DT)
    bddt = getattr(mybir.dt, _BD_DT)

    nc = bacc.Bacc("TRN2", target_bir_lowering=False, debug=False,
                   num_devices=NCORES)

    # ---------------- DRAM parameters ----------------
    xs = nc.dram_tensor("xs", [C, SLAB, W], mmdt, kind="ExternalInput")
    xsT = nc.dram_tensor("xsT", [SLAB * W, C], bddt, kind="ExternalInput")
    bndz = nc.dram_tensor("bndz", [NRING, 2, BROWS, BW], bddt)
    wcT = nc.dram_tensor("wcT", [C, CC], mmdt, kind="ExternalInput")
    bcc = nc.dram_tensor("bcc", [CC, 1], f32, kind="ExternalInput")
    weT = nc.dram_tensor("weT", [9, CC, CM], mmdt, kind="ExternalInput")
    be = nc.dram_tensor("be", [CM, 1], f32, kind="ExternalInput")
    sel01 = nc.dram_tensor("sel01", [CM, 4], mmdt, kind="ExternalInput")
    ident = nc.dram_tensor("ident", [128, 128], mmdt, kind="ExternalInput")
    out = nc.dram_tensor("out", [C, HO, WO], bddt, kind="ExternalOutput")

    COMP_W = W + 2      # 66: comp cols with 1 zero col each side

    with tile.TileContext(nc) as tc:
        with ExitStack() as ctx:
            cpool = ctx.enter_context(tc.tile_pool(name="const", bufs=1))
            xpool = ctx.enter_context(tc.tile_pool(name="xdata", bufs=1))
            work = ctx.enter_context(tc.tile_pool(name="work", bufs=3))
            opool = ctx.enter_context(tc.tile_pool(name="oevac", bufs=4))
            pers = ctx.enter_context(tc.tile_pool(name="pers", bufs=1))
            bpool = ctx.enter_context(tc.tile_pool(name="bndsb", bufs=3))
            ps_comp_p = ctx.enter_context(tc.tile_pool(name="pscomp", bufs=1, space="PSUM"))
            ps_mask_p = ctx.enter_context(tc.tile_pool(name="psmask", bufs=1, space="PSUM"))
            ps_z_p = ctx.enter_context(tc.tile_pool(name="psz", bufs=1, space="PSUM"))
            ps_t_p = ctx.enter_context(tc.tile_pool(name="pst", bufs=1, space="PSUM"))
            ps_o_p = ctx.enter_context(tc.tile_pool(name="pso", bufs=2, space="PSUM"))

            # ---------------- load constants (scalar queue) ----------------
            t_wc = [cpool.tile([128, CC], mmdt, tag=f"wc{k}", name=f"wc{k}") for k in range(2)]
            for k in range(2):
                nc.scalar.dma_start(t_wc[k][:], wcT.ap()[128 * k:128 * (k + 1), :])
            t_bcc = cpool.tile([CC, 1], f32, tag="bcc", name="bcc")
            nc.scalar.dma_start(t_bcc[:], bcc.ap())
            t_be = cpool.tile([CM, 1], f32, tag="be", name="be")
            nc.sync.dma_start(t_be[:], be.ap())
            t_sel = cpool.tile([CM, 4], mmdt, tag="sel", name="sel")
            nc.sync.dma_start(t_sel[:], sel01.ap())
            t_id = cpool.tile([128, 128], mmdt, tag="id", name="id")
            nc.sync.dma_start(t_id[:], ident.ap())

            # ---------------- load x (scalar queue, in slab-row chunks so
            # the comp conv can start before the whole load finishes) ------
            t_x = [xpool.tile([128, SLAB * W], mmdt, tag=f"x{k}", name=f"x{k}") for k in range(2)]
            cuts = [0, 10 * W, 22 * W, SLAB * W]

            def load_x_chunk(h):
                for k in range(2):
                    nc.scalar.dma_start(
                        t_x[k][:, cuts[h]:cuts[h + 1]],
                        xs.ap()[128 * k:128 * (k + 1), :, :]
                        .rearrange("p r w -> p (r w)")[:, cuts[h]:cuts[h + 1]])

            load_x_chunk(0)
            t_we = cpool.tile([CC, 9 * CM], mmdt, tag="we", name="we")
            src_we = AP(weT.ap().tensor, 0, [[CM, CC], [CC * CM, 9], [1, CM]])
            nc.scalar.dma_start(t_we[:], src_we)
            load_x_chunk(1)
            load_x_chunk(2)

            # xT: 35 overlapping row-pair tiles [128=(2 rows x 64 w), 256 c]
            # bf16.  Row s sits at partitions [0,64) of tile s AND [64,128)
            # of tile s-1, so both PE row-groups can be fed for row tiling.
            # All on the sync queue, which is otherwise idle during startup.
            t_xT = [xpool.tile([128, C], bddt, tag=f"xT{t}", name=f"xT{t}")
                    for t in range(SLAB - 1)]
            for t in range(SLAB - 1):
                nc.sync.dma_start(t_xT[t][:], xsT.ap()[W * t:W * t + 128, :])

            # ------------- zero the band ring once (gpsimd sw queue) -------
            t_zero = pers.tile([BROWS, BW], bddt, tag="zero", name="zero")
            nc.vector.memset(t_zero[:], 0.0)
            for s in range(NRING):
                for r in range(2):
                    nc.gpsimd.dma_start(bndz.ap()[s, r, :, :], t_zero[:])

            # ---------------- comp = 1x1 conv + bias (rows 1..34 of slab) ----
            # comp stored [CC, 34, 66] with zero cols 0 and 65
            t_comp = pers.tile([CC, (RH + 2) * COMP_W], mmdt, tag="comp", name="comp")
            compv = t_comp[:].rearrange("p (r w) -> p r w", w=COMP_W)
            nc.vector.memset(compv[:, :, 0:1].bitcast(f32), 0.0)
            nc.vector.memset(compv[:, :, COMP_W - 1:COMP_W].bitcast(f32), 0.0)

            NPIX_C = (RH + 2) * W  # 2176 pixels (rows 1..34 of slab)
            ctile = 512
            nct = (NPIX_C + ctile - 1) // ctile
            for nt in range(nct):
                p0 = nt * ctile
                n = min(ctile, NPIX_C - p0)
                ps = ps_comp_p.tile([CC, ctile], f32, tag="ps_comp", name="ps_comp")
                for k in range(2):
                    rhs = AP(t_x[k][:].tensor, W + p0, [[pstep(t_x[k]), 128], [1, n]])
                    nc.tensor.matmul(ps[:, :n], t_wc[k][:], rhs,
                                     start=(k == 0), stop=(k == 1))
                # evacuate into comp[., rows, 1:65]; bias added here
                r0 = p0 // W
                nr = n // W
                src_ps = ps[:, :n].rearrange("p (r w) -> p r w", w=W)
                nc.scalar.activation(compv[:, r0:r0 + nr, 1:1 + W], src_ps,
                                     func=AF.Identity, bias=t_bcc[:])

            # ---------------- mask conv 3x3 -> exp -> Z ----------------
            # emask [104, RH*W]: rows 0..99 exp(mask), rows 100..103 Z per ab
            t_em = pers.tile([CM + 4, PIXC], mmdt, tag="emask", name="emask")
            emv = t_em[:].rearrange("p (r w) -> p r w", w=W)
            mtile = 512
            for nt in range(PIXC // mtile):
                mr0 = nt * mtile // W   # 8 mask rows per tile
                ps = ps_mask_p.tile([CM, mtile], f32, tag="ps_mask", name="ps_mask")
                first = True
                for dy in range(3):
                    for dx in range(3):
                        tap = dy * 3 + dx
                        rhs = compv[:, mr0 + dy:mr0 + dy + 8, dx:dx + W]
                        nc.tensor.matmul(ps[:],
                                         t_we[:, tap * CM:(tap + 1) * CM],
                                         rhs, start=first,
                                         stop=(tap == 8))
                        first = False
                # exp(mask + be) -> emask rows 0..99
                dst = emv[0:CM, mr0:mr0 + 8, :]
                nc.scalar.activation(dst, ps[:].rearrange("p (r w) -> p r w", w=W),
                                     func=AF.Exp, bias=t_be[:])
                # Z = sel01.T @ emask chunk -> rows 100..104 (via staging DMA:
                # engine writes must start at partition 0/32/64/96)
                psz = ps_z_p.tile([4, mtile], f32, tag="ps_z", name="ps_z")
                nc.tensor.matmul(psz[:], t_sel[:], t_em[0:CM, nt * mtile:(nt + 1) * mtile],
                                 start=True, stop=True)
                t_zs = work.tile([4, mtile], mmdt, tag="zstage", name="zstage")
                nc.scalar.activation(t_zs[:], psz[:], func=AF.Copy)
                nc.gpsimd.dma_start(emv[CM:CM + 4, mr0:mr0 + 8, :],
                                    t_zs[:].rearrange("p (r w) -> p r w", w=W))

            # ---------------- per h-pair block ----------------
            for t in range(NBLK):
                slot = t % NRING
                # transpose emask[:, 2t:2t+2, :] -> [128 pix, 104]
                psT = ps_t_p.tile([128, CM + 4], mmdt, tag="ps_T", name="ps_T")
                src = emv[:, 2 * t:2 * t + 2, :].rearrange("p a b -> p (a b)")
                nc.tensor.transpose(psT[:], src, t_id[0:CM + 4, 0:CM + 4])
                t_emT = work.tile([128, CM + 4], f32, tag="emT", name="emT")
                nc.scalar.activation(t_emT[:], psT[:], func=AF.Copy)

                # reciprocal of Z (4 cols)
                t_rz = work.tile([128, 4], f32, tag="rz", name="rz")
                nc.vector.reciprocal(t_rz[:], t_emT[:, CM:CM + 4])

                # fused softmax-normalize + R'' permutation:
                # t_rp[:, 20q+10a+2p+b] = emT[:, 20p+4q+2a+b] * rz[:, 2a+b]
                t_rp = work.tile([128, CM], bddt, tag="rpp", name="rpp")
                rps = pstep(t_rp)
                emp = pstep(t_emT)
                rzp = pstep(t_rz)
                for a in range(2):
                    dstp = AP(t_rp[:].tensor, 10 * a,
                              [[rps, 128], [20, 5], [2, 5], [1, 2]])
                    in0 = AP(t_emT[:].tensor, 2 * a,
                             [[emp, 128], [4, 5], [20, 5], [1, 2]])
                    in1 = AP(t_rz[:].tensor, 2 * a,
                             [[rzp, 128], [0, 5], [0, 5], [1, 2]])
                    nc.vector.tensor_mul(dstp, in0, in1)

                # band scatter: 1 DMA per r, split across the two HW queues.
                # row = jl+q, col = 20jl+10a+2p+b
                with nc.allow_non_contiguous_dma(reason="banded mask scatter"):
                    for r in range(2):
                        dstb = AP(bndz.ap().tensor,
                                  (slot * 2 + r) * BROWS * BW,
                                  [[BW + 20, W], [BW, K5], [1, 20]])
                        srcb = AP(t_rp[:].tensor, (r * W) * rps,
                                  [[rps, W], [20, K5], [1, 20]])
                        eng = nc.sync if r == 0 else nc.scalar
                        eng.dma_start(dstb, srcb)

                # band readback: rows [2,66) of each r section; r=1 lands in
                # partitions [64,128) so its matmuls run in PE row-group 2-3
                # (row tiling) concurrently with r=0 in row-group 0-1.
                bnd = bpool.tile([128, BW], bddt, tag="bnd", name="bnd")
                bps = pstep(bnd)
                for r in range(2):
                    srcr = AP(bndz.ap().tensor,
                              (slot * 2 + r) * BROWS * BW + 2 * BW,
                              [[BW, W], [1, BW]])
                    eng = nc.sync if r == 0 else nc.scalar
                    eng.dma_start(bnd[64 * r:64 * (r + 1), :], srcr)

                # CARAFE: psum[c, (a,jl,b)] += xT_row.T @ band_p
                for ct in range(2):
                    pso = [ps_o_p.tile([128, 256], f32, tag=f"ps_o{r}",
                                       name=f"ps_o{r}") for r in range(2)]
                    for p in range(K5):
                        for r in range(2):
                            rhs = AP(bnd[:].tensor, 64 * r * bps + 2 * p,
                                     [[bps, W], [10, 2], [20, W], [1, 2]])
                            nc.tensor.matmul(
                                pso[r][:],
                                t_xT[2 * t + p][64 * r:64 * (r + 1),
                                                128 * ct:128 * (ct + 1)],
                                rhs, start=(p == 0), stop=(p == K5 - 1))
                    t_o = opool.tile([128, 512], bddt, tag=f"osb{ct}", name=f"osb{ct}")
                    for r in range(2):
                        nc.vector.tensor_copy(t_o[:, 256 * r:256 * (r + 1)], pso[r][:])
                    # DMA to out[c, hr, j]: cols (r,a,jl,b) = 4 contiguous hr rows
                    dsto = AP(out.ap().tensor,
                              ct * 128 * HO * WO + 4 * t * WO,
                              [[HO * WO, 128], [1, 512]])
                    eng = nc.scalar if ct == 0 else nc.sync
                    eng.dma_start(dsto, t_o[:])

    nc.compile()
    return nc


_CACHE = {}


def _get_program():
    if "nc" not in _CACHE:
        _CACHE["nc"] = _build_program()
    return _CACHE["nc"]


def host_prep(x, w_comp, b_comp, w_enc, b_enc):
    """Build per-core input maps."""
    import ml_dtypes
    bf16 = ml_dtypes.bfloat16
    x = np.asarray(x, dtype=np.float32)
    wcT = np.ascontiguousarray(np.asarray(w_comp, np.float32).reshape(CC, C).T)
    bcc = np.asarray(b_comp, np.float32).reshape(CC, 1)
    # weT[tap, cin, cout]
    weT = np.ascontiguousarray(
        np.asarray(w_enc, np.float32).reshape(CM, CC, 9).transpose(2, 1, 0))
    be = np.asarray(b_enc, np.float32).reshape(CM, 1)
    sel = np.zeros((CM, 4), np.float32)
    sel[np.arange(CM), np.arange(CM) % 4] = 1.0
    ident = np.eye(128, dtype=np.float32)

    in_maps = []
    for core in range(NCORES):
        n, half = core // 2, core % 2
        h0 = RH * half
        slab = np.zeros((C, SLAB, W), np.float32)
        r_lo, r_hi = h0 - 2, h0 + SLAB - 2       # x rows [r_lo, r_hi)
        v_lo, v_hi = max(0, r_lo), min(H, r_hi)
        slab[:, v_lo - r_lo:v_hi - r_lo, :] = x[n, :, v_lo:v_hi, :]
        xsT = np.ascontiguousarray(
            slab.reshape(C, SLAB * W).T).astype(bf16)
        in_maps.append({"xs": slab, "xsT": xsT, "wcT": wcT, "bcc": bcc,
                        "weT": weT, "be": be, "sel01": sel, "ident": ident})
    return in_maps


def host_gather(results):
    out = np.empty((N_B, C, S * H, S * W), np.float32)
    for core in range(NCORES):
        n, half = core // 2, core % 2
        out[n, :, HO * half:HO * (half + 1), :] = \
            results[core]["out"].astype(np.float32)
    return out


def kernel(x, w_comp, b_comp, w_enc, b_enc):
    from concourse.bass_utils import run_bass_kernel_spmd
    nc = _get_program()
    in_maps = host_prep(x, w_comp, b_comp, w_enc, b_enc)
    res = run_bass_kernel_spmd(nc, in_maps, list(range(NCORES)))
    return host_gather(res.results)


# revision 33
# speedup vs baseline: 1.2033x; 1.0369x over previous
"""CARAFE (content-aware upsampling) Trainium2 kernel.

Full inputs -> shard over 8 NeuronCores (batch x image-half) -> bass/Tile
kernel per core -> gather full output.

Reference semantics:
  comp = conv1x1(x, w_comp) + b_comp                    [n,64,64,64]
  mask = conv3x3(comp, w_enc, pad=1) + b_enc            [n,100,64,64]
  m    = softmax over 25 of pixel_shuffle(mask, 2)      [n,25,128,128]
  out[n,c,i,j] = sum_k m[n,k,i,j] * xpad[n,c,i//2+p, j//2+q],  k=5p+q

Band construction (v3): per h-pair block and low-res row r, mask values
live in a DRAM "band" image of 68 rows x 1280 cols:
  row = jl + q   (rows 0,1,66,67 are pad; valid w' = row-2)
  col = 20*jl + 10*a + 2*p + b
Nonzero cells of each row form one contiguous 100-el run, so the
SBUF->DRAM scatter is 1 flat shear DMA per (block, r).  Bands are read
back as [64 w', 1280] tiles and contracted against per-row xT tiles on
the PE array.  4-slot ring, zeroed once; slot cells are overwritten in
place each block so no re-zeroing is needed.
"""
import numpy as np
import sys
from contextlib import ExitStack

sys.path.insert(0, "/opt/trn_rl_repo")

# ---------------- problem constants (hardcoded per spec) ----------------
N_B, C, H, W = 4, 256, 64, 64
CC = 64            # compressed channels
K5 = 5             # carafe kernel
S = 2              # scale
CM = K5 * K5 * S * S   # 100 mask channels
NCORES = 8
RH = H // 2        # 32 low-res rows per core
SLAB = RH + 4      # 36 x-rows per core (h0-2 .. h0+33)
NBLK = RH // 2     # 16 h-pair blocks
PIXC = RH * W      # 2048 low-res pixels per core
HO, WO = 2 * RH, 2 * W   # 64 x 128 output shard

NRING = 16         # band ring slots (one per block)
BW = 20 * W        # 1280 band cols: (jl,a,p,b)
BROWS = W + 4      # 68 band rows: jl + q, 2 pad rows each side

_MM_DT = "float32r"   # comp/mask conv matmul dtype
_BD_DT = "bfloat16"   # carafe path: xT, softmax mask, band, output


def _build_program():
    import concourse.bass as bass
    import concourse.tile as tile
    from concourse import bacc, mybir
    from concourse.ap import AP
    AF = mybir.ActivationFunctionType

    def pstep(t):
        return t[:].ap[0][0]

    f32 = mybir.dt.float32
    mmdt = getattr(mybir.dt, _MM_DT)
    bddt = getattr(mybir.dt, _BD_DT)

    nc = bacc.Bacc("TRN2", target_bir_lowering=False, debug=False,
                   num_devices=NCORES)

    # ---------------- DRAM parameters ----------------
    xs = nc.dram_tensor("xs", [C, SLAB, W], mmdt, kind="ExternalInput")
    xsT = nc.dram_tensor("xsT", [SLAB * W, C], bddt, kind="ExternalInput")
    bndz = nc.dram_tensor("bndz", [NRING, 2, BROWS, BW], bddt)
    wcT = nc.dram_tensor("wcT", [C, CC], mmdt, kind="ExternalInput")
    bcc = nc.dram_tensor("bcc", [CC, 1], f32, kind="ExternalInput")
    weT = nc.dram_tensor("weT", [9, CC, CM], mmdt, kind="ExternalInput")
    be = nc.dram_tensor("be", [CM, 1], f32, kind="ExternalInput")
    sel01 = nc.dram_tensor("sel01", [CM, 4], mmdt, kind="ExternalInput")
    ident = nc.dram_tensor("ident", [128, 128], mmdt, kind="ExternalInput")
    out = nc.dram_tensor("out", [C, HO, WO], bddt, kind="ExternalOutput")

    COMP_W = W + 2      # 66: comp cols with 1 zero col each side

    with tile.TileContext(nc) as tc:
        with ExitStack() as ctx:
            cpool = ctx.enter_context(tc.tile_pool(name="const", bufs=1))
            xpool = ctx.enter_context(tc.tile_pool(name="xdata", bufs=1))
            work = ctx.enter_context(tc.tile_pool(name="work", bufs=3))
            opool = ctx.enter_context(tc.tile_pool(name="oevac", bufs=4))
            pers = ctx.enter_context(tc.tile_pool(name="pers", bufs=1))
            bpool = ctx.enter_context(tc.tile_pool(name="bndsb", bufs=3))
            ps_comp_p = ctx.enter_context(tc.tile_pool(name="pscomp", bufs=1, space="PSUM"))
            ps_mask_p = ctx.enter_context(tc.tile_pool(name="psmask", bufs=1, space="PSUM"))
            ps_z_p = ctx.enter_context(tc.tile_pool(name="psz", bufs=1, space="PSUM"))
            ps_t_p = ctx.enter_context(tc.tile_pool(name="pst", bufs=1, space="PSUM"))
            ps_o_p = ctx.enter_context(tc.tile_pool(name="pso", bufs=2, space="PSUM"))

            # ---------------- load constants (scalar queue) ----------------
            t_wc = [cpool.tile([128, CC], mmdt, tag=f"wc{k}", name=f"wc{k}") for k in range(2)]
            for k in range(2):
                nc.scalar.dma_start(t_wc[k][:], wcT.ap()[128 * k:128 * (k + 1), :])
            t_bcc = cpool.tile([CC, 1], f32, tag="bcc", name="bcc")
            nc.scalar.dma_start(t_bcc[:], bcc.ap())
            t_be = cpool.tile([CM, 1], f32, tag="be", name="be")
            nc.sync.dma_start(t_be[:], be.ap())
            t_sel = cpool.tile([CM, 4], mmdt, tag="sel", name="sel")
            nc.sync.dma_start(t_sel[:], sel01.ap())
            t_id = cpool.tile([128, 128], mmdt, tag="id", name="id")
            nc.sync.dma_start(t_id[:], ident.ap())

            # ---------------- load x (scalar queue, in slab-row chunks so
            # the comp conv can start before the whole load finishes) ------
            t_x = [xpool.tile([128, SLAB * W], mmdt, tag=f"x{k}", name=f"x{k}") for k in range(2)]
            cuts = [0, 10 * W, 22 * W, SLAB * W]

            def load_x_chunk(h, eng):
                for k in range(2):
                    eng.dma_start(
                        t_x[k][:, cuts[h]:cuts[h + 1]],
                        xs.ap()[128 * k:128 * (k + 1), :, :]
                        .rearrange("p r w -> p (r w)")[:, cuts[h]:cuts[h + 1]])

            load_x_chunk(0, nc.scalar)
            load_x_chunk(1, nc.sync)
            t_we = cpool.tile([CC, 9 * CM], mmdt, tag="we", name="we")
            src_we = AP(weT.ap().tensor, 0, [[CM, CC], [CC * CM, 9], [1, CM]])
            nc.scalar.dma_start(t_we[:], src_we)
            load_x_chunk(2, nc.scalar)

            # xT: 35 overlapping row-pair tiles [128=(2 rows x 64 w), 256 c]
            # bf16.  Row s sits at partitions [0,64) of tile s AND [64,128)
            # of tile s-1, so both PE row-groups can be fed for row tiling.
            # All on the sync queue, which is otherwise idle during startup.
            t_xT = [xpool.tile([128, C], bddt, tag=f"xT{t}", name=f"xT{t}")
                    for t in range(SLAB - 1)]
            for t in range(SLAB - 1):
                nc.sync.dma_start(t_xT[t][:], xsT.ap()[W * t:W * t + 128, :])

            # ------------- zero the band ring once (gpsimd sw queue) -------
            t_zero = pers.tile([BROWS, BW], bddt, tag="zero", name="zero")
            nc.vector.memset(t_zero[:], 0.0)
            for s in range(NRING):
                for r in range(2):
                    nc.gpsimd.dma_start(bndz.ap()[s, r, :, :], t_zero[:])

            # ---------------- comp = 1x1 conv + bias (rows 1..34 of slab) ----
            # comp stored [CC, 34, 66] with zero cols 0 and 65
            t_comp = pers.tile([CC, (RH + 2) * COMP_W], mmdt, tag="comp", name="comp")
            compv = t_comp[:].rearrange("p (r w) -> p r w", w=COMP_W)
            nc.vector.memset(compv[:, :, 0:1].bitcast(f32), 0.0)
            nc.vector.memset(compv[:, :, COMP_W - 1:COMP_W].bitcast(f32), 0.0)

            NPIX_C = (RH + 2) * W  # 2176 pixels (rows 1..34 of slab)
            ctile = 512
            nct = (NPIX_C + ctile - 1) // ctile
            for nt in range(nct):
                p0 = nt * ctile
                n = min(ctile, NPIX_C - p0)
                ps = ps_comp_p.tile([CC, ctile], f32, tag="ps_comp", name="ps_comp")
                for k in range(2):
                    rhs = AP(t_x[k][:].tensor, W + p0, [[pstep(t_x[k]), 128], [1, n]])
                    nc.tensor.matmul(ps[:, :n], t_wc[k][:], rhs,
                                     start=(k == 0), stop=(k == 1))
                # evacuate into comp[., rows, 1:65]; bias added here
                r0 = p0 // W
                nr = n // W
                src_ps = ps[:, :n].rearrange("p (r w) -> p r w", w=W)
                nc.scalar.activation(compv[:, r0:r0 + nr, 1:1 + W], src_ps,
                                     func=AF.Identity, bias=t_bcc[:])

            # ---------------- mask conv 3x3 -> exp -> Z ----------------
            # emask [104, RH*W]: rows 0..99 exp(mask), rows 100..103 Z per ab
            t_em = pers.tile([CM + 4, PIXC], mmdt, tag="emask", name="emask")
            emv = t_em[:].rearrange("p (r w) -> p r w", w=W)
            mtile = 512
            for nt in range(PIXC // mtile):
                mr0 = nt * mtile // W   # 8 mask rows per tile
                ps = ps_mask_p.tile([CM, mtile], f32, tag="ps_mask", name="ps_mask")
                first = True
                for dy in range(3):
                    for dx in range(3):
                        tap = dy * 3 + dx
                        rhs = compv[:, mr0 + dy:mr0 + dy + 8, dx:dx + W]
                        nc.tensor.matmul(ps[:],
                                         t_we[:, tap * CM:(tap + 1) * CM],
                                         rhs, start=first,
                                         stop=(tap == 8))
                        first = False
                # exp(mask + be) -> emask rows 0..99
                dst = emv[0:CM, mr0:mr0 + 8, :]
                nc.scalar.activation(dst, ps[:].rearrange("p (r w) -> p r w", w=W),
                                     func=AF.Exp, bias=t_be[:])
                # Z = sel01.T @ emask chunk -> rows 100..104 (via staging DMA:
                # engine writes must start at partition 0/32/64/96)
                psz = ps_z_p.tile([4, mtile], f32, tag="ps_z", name="ps_z")
                nc.tensor.matmul(psz[:], t_sel[:], t_em[0:CM, nt * mtile:(nt + 1) * mtile],
                                 start=True, stop=True)
                t_zs = work.tile([4, mtile], mmdt, tag="zstage", name="zstage")
                nc.scalar.activation(t_zs[:], psz[:], func=AF.Copy)
                nc.gpsimd.dma_start(emv[CM:CM + 4, mr0:mr0 + 8, :],
                                    t_zs[:].rearrange("p (r w) -> p r w", w=W))

            # ------------- mask side, hoisted for all blocks ---------------
            # transpose -> normalize+permute -> scatter, one band slot per
            # block.  Runs as soon as each mask tile is ready, well before
            # the CARAFE stream needs the band.
            for t in range(NBLK):
                slot = t % NRING
                # transpose emask[:, 2t:2t+2, :] -> [128 pix, 104]
                psT = ps_t_p.tile([128, CM + 4], mmdt, tag="ps_T", name="ps_T")
                src = emv[:, 2 * t:2 * t + 2, :].rearrange("p a b -> p (a b)")
                nc.tensor.transpose(psT[:], src, t_id[0:CM + 4, 0:CM + 4])
                t_emT = work.tile([128, CM + 4], f32, tag="emT", name="emT")
                nc.scalar.activation(t_emT[:], psT[:], func=AF.Copy)

                # reciprocal of Z (4 cols)
                t_rz = work.tile([128, 4], f32, tag="rz", name="rz")
                nc.vector.reciprocal(t_rz[:], t_emT[:, CM:CM + 4])

                # fused softmax-normalize + R'' permutation:
                # t_rp[:, 20q+10a+2p+b] = emT[:, 20p+4q+2a+b] * rz[:, 2a+b]
                t_rp = work.tile([128, CM], bddt, tag="rpp", name="rpp")
                rps = pstep(t_rp)
                emp = pstep(t_emT)
                rzp = pstep(t_rz)
                for a in range(2):
                    dstp = AP(t_rp[:].tensor, 10 * a,
                              [[rps, 128], [20, 5], [2, 5], [1, 2]])
                    in0 = AP(t_emT[:].tensor, 2 * a,
                             [[emp, 128], [4, 5], [20, 5], [1, 2]])
                    in1 = AP(t_rz[:].tensor, 2 * a,
                             [[rzp, 128], [0, 5], [0, 5], [1, 2]])
                    nc.vector.tensor_mul(dstp, in0, in1)

                # band scatter: 1 DMA per r, split across the two HW queues.
                # row = jl+q, col = 20jl+10a+2p+b
                with nc.allow_non_contiguous_dma(reason="banded mask scatter"):
                    for r in range(2):
                        dstb = AP(bndz.ap().tensor,
                                  (slot * 2 + r) * BROWS * BW,
                                  [[BW + 20, W], [BW, K5], [1, 20]])
                        srcb = AP(t_rp[:].tensor, (r * W) * rps,
                                  [[rps, W], [20, K5], [1, 20]])
                        eng = nc.sync if r == 0 else nc.scalar
                        eng.dma_start(dstb, srcb)

            # ---------------- CARAFE stream, per block ---------------------
            for t in range(NBLK):
                slot = t % NRING
                # band readback: rows [2,66) of each r section; r=1 lands in
                # partitions [64,128) so its matmuls run in PE row-group 2-3
                # (row tiling) concurrently with r=0 in row-group 0-1.
                bnd = bpool.tile([128, BW], bddt, tag="bnd", name="bnd")
                bps = pstep(bnd)
                for r in range(2):
                    srcr = AP(bndz.ap().tensor,
                              (slot * 2 + r) * BROWS * BW + 2 * BW,
                              [[BW, W], [1, BW]])
                    eng = nc.sync if r == 0 else nc.scalar
                    eng.dma_start(bnd[64 * r:64 * (r + 1), :], srcr)

                # CARAFE: psum[c, (a,jl,b)] += xT_row.T @ band_p
                for ct in range(2):
                    pso = [ps_o_p.tile([128, 256], f32, tag=f"ps_o{r}",
                                       name=f"ps_o{r}") for r in range(2)]
                    for p in range(K5):
                        for r in range(2):
                            rhs = AP(bnd[:].tensor, 64 * r * bps + 2 * p,
                                     [[bps, W], [10, 2], [20, W], [1, 2]])
                            nc.tensor.matmul(
                                pso[r][:],
                                t_xT[2 * t + p][64 * r:64 * (r + 1),
                                                128 * ct:128 * (ct + 1)],
                                rhs, start=(p == 0), stop=(p == K5 - 1))
                    t_o = opool.tile([128, 512], bddt, tag=f"osb{ct}", name=f"osb{ct}")
                    for r in range(2):
                        nc.vector.tensor_copy(t_o[:, 256 * r:256 * (r + 1)], pso[r][:])
                    # DMA to out[c, hr, j]: cols (r,a,jl,b) = 4 contiguous hr rows
                    dsto = AP(out.ap().tensor,
                              ct * 128 * HO * WO + 4 * t * WO,
                              [[HO * WO, 128], [1, 512]])
                    eng = nc.scalar if ct == 0 else nc.sync
                    eng.dma_start(dsto, t_o[:])

    nc.compile()
    return nc


_CACHE = {}


def _get_program():
    if "nc" not in _CACHE:
        _CACHE["nc"] = _build_program()
    return _CACHE["nc"]


def host_prep(x, w_comp, b_comp, w_enc, b_enc):
    """Build per-core input maps."""
    import ml_dtypes
    bf16 = ml_dtypes.bfloat16
    x = np.asarray(x, dtype=np.float32)
    wcT = np.ascontiguousarray(np.asarray(w_comp, np.float32).reshape(CC, C).T)
    bcc = np.asarray(b_comp, np.float32).reshape(CC, 1)
    # weT[tap, cin, cout]
    weT = np.ascontiguousarray(
        np.asarray(w_enc, np.float32).reshape(CM, CC, 9).transpose(2, 1, 0))
    be = np.asarray(b_enc, np.float32).reshape(CM, 1)
    sel = np.zeros((CM, 4), np.float32)
    sel[np.arange(CM), np.arange(CM) % 4] = 1.0
    ident = np.eye(128, dtype=np.float32)

    in_maps = []
    for core in range(NCORES):
        n, half = core // 2, core % 2
        h0 = RH * half
        slab = np.zeros((C, SLAB, W), np.float32)
        r_lo, r_hi = h0 - 2, h0 + SLAB - 2       # x rows [r_lo, r_hi)
        v_lo, v_hi = max(0, r_lo), min(H, r_hi)
        slab[:, v_lo - r_lo:v_hi - r_lo, :] = x[n, :, v_lo:v_hi, :]
        xsT = np.ascontiguousarray(
            slab.reshape(C, SLAB * W).T).astype(bf16)
        in_maps.append({"xs": slab, "xsT": xsT, "wcT": wcT, "bcc": bcc,
                        "weT": weT, "be": be, "sel01": sel, "ident": ident})
    return in_maps


def host_gather(results):
    out = np.empty((N_B, C, S * H, S * W), np.float32)
    for core in range(NCORES):
        n, half = core // 2, core % 2
        out[n, :, HO * half:HO * (half + 1), :] = \
            results[core]["out"].astype(np.float32)
    return out


def kernel(x, w_comp, b_comp, w_enc, b_enc):
    from concourse.bass_utils import run_bass_kernel_spmd
    nc = _get_program()
    in_maps = host_prep(x, w_comp, b_comp, w_enc, b_enc)
    res = run_bass_kernel_spmd(nc, in_maps, list(range(NCORES)))
    return host_gather(res.results)


# revision 35
# speedup vs baseline: 1.2121x; 1.0073x over previous
"""CARAFE (content-aware upsampling) Trainium2 kernel.

Full inputs -> shard over 8 NeuronCores (batch x image-half) -> bass/Tile
kernel per core -> gather full output.

Reference semantics:
  comp = conv1x1(x, w_comp) + b_comp                    [n,64,64,64]
  mask = conv3x3(comp, w_enc, pad=1) + b_enc            [n,100,64,64]
  m    = softmax over 25 of pixel_shuffle(mask, 2)      [n,25,128,128]
  out[n,c,i,j] = sum_k m[n,k,i,j] * xpad[n,c,i//2+p, j//2+q],  k=5p+q

Band construction (v3): per h-pair block and low-res row r, mask values
live in a DRAM "band" image of 68 rows x 1280 cols:
  row = jl + q   (rows 0,1,66,67 are pad; valid w' = row-2)
  col = 20*jl + 10*a + 2*p + b
Nonzero cells of each row form one contiguous 100-el run, so the
SBUF->DRAM scatter is 1 flat shear DMA per (block, r).  Bands are read
back as [64 w', 1280] tiles and contracted against per-row xT tiles on
the PE array.  16-slot ring, zeroed once via the gpsimd software DMA
queue; slot cells are fully overwritten each pass so no re-zeroing is
needed.
"""
import numpy as np
import sys
from contextlib import ExitStack

sys.path.insert(0, "/opt/trn_rl_repo")

# ---------------- problem constants (hardcoded per spec) ----------------
N_B, C, H, W = 4, 256, 64, 64
CC = 64            # compressed channels
K5 = 5             # carafe kernel
S = 2              # scale
CM = K5 * K5 * S * S   # 100 mask channels
NCORES = 8
RH = H // 2        # 32 low-res rows per core
SLAB = RH + 4      # 36 x-rows per core (h0-2 .. h0+33)
NBLK = RH // 2     # 16 h-pair blocks
PIXC = RH * W      # 2048 low-res pixels per core
HO, WO = 2 * RH, 2 * W   # 64 x 128 output shard

NRING = 16         # band ring slots (one per block)
BW = 20 * W        # 1280 band cols: (jl,a,p,b)
BROWS = W + 4      # 68 band rows: jl + q, 2 pad rows each side

_MM_DT = "float32r"   # comp/mask conv matmul dtype
_BD_DT = "bfloat16"   # carafe path: xT, softmax mask, band, output


def _build_program():
    import concourse.bass as bass
    import concourse.tile as tile
    from concourse import bacc, mybir
    from concourse.ap import AP
    AF = mybir.ActivationFunctionType

    def pstep(t):
        return t[:].ap[0][0]

    f32 = mybir.dt.float32
    mmdt = getattr(mybir.dt, _MM_DT)
    bddt = getattr(mybir.dt, _BD_DT)

    nc = bacc.Bacc("TRN2", target_bir_lowering=False, debug=False,
                   num_devices=NCORES)

    # ---------------- DRAM parameters ----------------
    xs = nc.dram_tensor("xs", [C, SLAB, W], mmdt, kind="ExternalInput")
    xsT = nc.dram_tensor("xsT", [SLAB * W, C], bddt, kind="ExternalInput")
    bndz = nc.dram_tensor("bndz", [NRING, 2, BROWS, BW], bddt)
    wcT = nc.dram_tensor("wcT", [C, CC], mmdt, kind="ExternalInput")
    bcc = nc.dram_tensor("bcc", [CC, 1], f32, kind="ExternalInput")
    weT = nc.dram_tensor("weT", [9, CC, CM], mmdt, kind="ExternalInput")
    be = nc.dram_tensor("be", [CM, 1], f32, kind="ExternalInput")
    sel01 = nc.dram_tensor("sel01", [CM, 4], mmdt, kind="ExternalInput")
    ident = nc.dram_tensor("ident", [128, 128], mmdt, kind="ExternalInput")
    out = nc.dram_tensor("out", [C, HO, WO], bddt, kind="ExternalOutput")

    COMP_W = W + 2      # 66: comp cols with 1 zero col each side

    with tile.TileContext(nc) as tc:
        with ExitStack() as ctx:
            cpool = ctx.enter_context(tc.tile_pool(name="const", bufs=1))
            xpool = ctx.enter_context(tc.tile_pool(name="xdata", bufs=1))
            work = ctx.enter_context(tc.tile_pool(name="work", bufs=3))
            opool = ctx.enter_context(tc.tile_pool(name="oevac", bufs=6))
            pers = ctx.enter_context(tc.tile_pool(name="pers", bufs=1))
            bpool = ctx.enter_context(tc.tile_pool(name="bndsb", bufs=5))
            ps_comp_p = ctx.enter_context(tc.tile_pool(name="pscomp", bufs=1, space="PSUM"))
            ps_mask_p = ctx.enter_context(tc.tile_pool(name="psmask", bufs=1, space="PSUM"))
            ps_z_p = ctx.enter_context(tc.tile_pool(name="psz", bufs=1, space="PSUM"))
            ps_t_p = ctx.enter_context(tc.tile_pool(name="pst", bufs=1, space="PSUM"))
            ps_o_p = ctx.enter_context(tc.tile_pool(name="pso", bufs=2, space="PSUM"))

            # ---------------- load constants (scalar queue) ----------------
            t_wc = [cpool.tile([128, CC], mmdt, tag=f"wc{k}", name=f"wc{k}") for k in range(2)]
            for k in range(2):
                nc.scalar.dma_start(t_wc[k][:], wcT.ap()[128 * k:128 * (k + 1), :])
            t_bcc = cpool.tile([CC, 1], f32, tag="bcc", name="bcc")
            nc.scalar.dma_start(t_bcc[:], bcc.ap())
            t_be = cpool.tile([CM, 1], f32, tag="be", name="be")
            nc.sync.dma_start(t_be[:], be.ap())
            t_sel = cpool.tile([CM, 4], mmdt, tag="sel", name="sel")
            nc.sync.dma_start(t_sel[:], sel01.ap())
            t_id = cpool.tile([128, 128], mmdt, tag="id", name="id")
            nc.sync.dma_start(t_id[:], ident.ap())

            # ---------------- load x (scalar queue, in slab-row chunks so
            # the comp conv can start before the whole load finishes) ------
            t_x = [xpool.tile([128, SLAB * W], mmdt, tag=f"x{k}", name=f"x{k}") for k in range(2)]
            cuts = [0, 10 * W, 22 * W, SLAB * W]

            def load_x_chunk(h, eng):
                for k in range(2):
                    eng.dma_start(
                        t_x[k][:, cuts[h]:cuts[h + 1]],
                        xs.ap()[128 * k:128 * (k + 1), :, :]
                        .rearrange("p r w -> p (r w)")[:, cuts[h]:cuts[h + 1]])

            load_x_chunk(0, nc.scalar)
            load_x_chunk(1, nc.scalar)
            t_we = cpool.tile([CC, 9 * CM], mmdt, tag="we", name="we")
            src_we = AP(weT.ap().tensor, 0, [[CM, CC], [CC * CM, 9], [1, CM]])
            nc.gpsimd.dma_start(t_we[:], src_we)
            load_x_chunk(2, nc.gpsimd)

            # xT: 35 overlapping row-pair tiles [128=(2 rows x 64 w), 256 c]
            # bf16.  Row s sits at partitions [0,64) of tile s AND [64,128)
            # of tile s-1, so both PE row-groups can be fed for row tiling.
            # All on the sync queue, which is otherwise idle during startup.
            t_xT = [xpool.tile([128, C], bddt, tag=f"xT{t}", name=f"xT{t}")
                    for t in range(SLAB - 1)]
            for t in range(SLAB - 1):
                nc.sync.dma_start(t_xT[t][:], xsT.ap()[W * t:W * t + 128, :])

            # ------------- zero the band ring once (gpsimd sw queue) -------
            t_zero = pers.tile([BROWS, BW], bddt, tag="zero", name="zero")
            nc.vector.memset(t_zero[:], 0.0)
            for s in range(NRING):
                for r in range(2):
                    nc.gpsimd.dma_start(bndz.ap()[s, r, :, :], t_zero[:])

            # ---------------- comp = 1x1 conv + bias (rows 1..34 of slab) ----
            # comp stored [CC, 34, 66] with zero cols 0 and 65
            t_comp = pers.tile([CC, (RH + 2) * COMP_W], mmdt, tag="comp", name="comp")
            compv = t_comp[:].rearrange("p (r w) -> p r w", w=COMP_W)
            nc.vector.memset(compv[:, :, 0:1].bitcast(f32), 0.0)
            nc.vector.memset(compv[:, :, COMP_W - 1:COMP_W].bitcast(f32), 0.0)

            NPIX_C = (RH + 2) * W  # 2176 pixels (rows 1..34 of slab)
            ctile = 512
            nct = (NPIX_C + ctile - 1) // ctile
            for nt in range(nct):
                p0 = nt * ctile
                n = min(ctile, NPIX_C - p0)
                ps = ps_comp_p.tile([CC, ctile], f32, tag="ps_comp", name="ps_comp")
                for k in range(2):
                    rhs = AP(t_x[k][:].tensor, W + p0, [[pstep(t_x[k]), 128], [1, n]])
                    nc.tensor.matmul(ps[:, :n], t_wc[k][:], rhs,
                                     start=(k == 0), stop=(k == 1))
                # evacuate into comp[., rows, 1:65]; bias added here
                r0 = p0 // W
                nr = n // W
                src_ps = ps[:, :n].rearrange("p (r w) -> p r w", w=W)
                nc.scalar.activation(compv[:, r0:r0 + nr, 1:1 + W], src_ps,
                                     func=AF.Identity, bias=t_bcc[:])

            # ---------------- mask conv 3x3 -> exp -> Z ----------------
            # emask [104, RH*W]: rows 0..99 exp(mask), rows 100..103 Z per ab
            t_em = pers.tile([CM + 4, PIXC], mmdt, tag="emask", name="emask")
            emv = t_em[:].rearrange("p (r w) -> p r w", w=W)
            mtile = 512
            for nt in range(PIXC // mtile):
                mr0 = nt * mtile // W   # 8 mask rows per tile
                ps = ps_mask_p.tile([CM, mtile], f32, tag="ps_mask", name="ps_mask")
                first = True
                for dy in range(3):
                    for dx in range(3):
                        tap = dy * 3 + dx
                        rhs = compv[:, mr0 + dy:mr0 + dy + 8, dx:dx + W]
                        nc.tensor.matmul(ps[:],
                                         t_we[:, tap * CM:(tap + 1) * CM],
                                         rhs, start=first,
                                         stop=(tap == 8))
                        first = False
                # exp(mask + be) -> emask rows 0..99
                dst = emv[0:CM, mr0:mr0 + 8, :]
                nc.scalar.activation(dst, ps[:].rearrange("p (r w) -> p r w", w=W),
                                     func=AF.Exp, bias=t_be[:])
                # Z = sel01.T @ emask chunk -> rows 100..104 (via staging DMA:
                # engine writes must start at partition 0/32/64/96)
                psz = ps_z_p.tile([4, mtile], f32, tag="ps_z", name="ps_z")
                nc.tensor.matmul(psz[:], t_sel[:], t_em[0:CM, nt * mtile:(nt + 1) * mtile],
                                 start=True, stop=True)
                t_zs = work.tile([4, mtile], mmdt, tag="zstage", name="zstage")
                nc.scalar.activation(t_zs[:], psz[:], func=AF.Copy)
                nc.gpsimd.dma_start(emv[CM:CM + 4, mr0:mr0 + 8, :],
                                    t_zs[:].rearrange("p (r w) -> p r w", w=W))

            # ------------- mask side, hoisted for all blocks ---------------
            # transpose -> normalize+permute -> scatter, one band slot per
            # block.  Runs as soon as each mask tile is ready, well before
            # the CARAFE stream needs the band.
            for t in range(NBLK):
                slot = t % NRING
                # transpose emask[:, 2t:2t+2, :] -> [128 pix, 104]
                psT = ps_t_p.tile([128, CM + 4], mmdt, tag="ps_T", name="ps_T")
                src = emv[:, 2 * t:2 * t + 2, :].rearrange("p a b -> p (a b)")
                nc.tensor.transpose(psT[:], src, t_id[0:CM + 4, 0:CM + 4])
                t_emT = work.tile([128, CM + 4], f32, tag="emT", name="emT")
                nc.scalar.activation(t_emT[:], psT[:], func=AF.Copy)

                # reciprocal of Z (4 cols)
                t_rz = work.tile([128, 4], f32, tag="rz", name="rz")
                nc.vector.reciprocal(t_rz[:], t_emT[:, CM:CM + 4])

                # fused softmax-normalize + R'' permutation:
                # t_rp[:, 20q+10a+2p+b] = emT[:, 20p+4q+2a+b] * rz[:, 2a+b]
                t_rp = work.tile([128, CM], bddt, tag="rpp", name="rpp")
                rps = pstep(t_rp)
                emp = pstep(t_emT)
                rzp = pstep(t_rz)
                for a in range(2):
                    dstp = AP(t_rp[:].tensor, 10 * a,
                              [[rps, 128], [20, 5], [2, 5], [1, 2]])
                    in0 = AP(t_emT[:].tensor, 2 * a,
                             [[emp, 128], [4, 5], [20, 5], [1, 2]])
                    in1 = AP(t_rz[:].tensor, 2 * a,
                             [[rzp, 128], [0, 5], [0, 5], [1, 2]])
                    nc.vector.tensor_mul(dstp, in0, in1)

                # band scatter: 1 DMA per r, split across the two HW queues.
                # row = jl+q, col = 20jl+10a+2p+b
                with nc.allow_non_contiguous_dma(reason="banded mask scatter"):
                    for r in range(2):
                        dstb = AP(bndz.ap().tensor,
                                  (slot * 2 + r) * BROWS * BW,
                                  [[BW + 20, W], [BW, K5], [1, 20]])
                        srcb = AP(t_rp[:].tensor, (r * W) * rps,
                                  [[rps, W], [20, K5], [1, 20]])
                        eng = nc.sync if r == 0 else nc.scalar
                        eng.dma_start(dstb, srcb)

            # ---------------- CARAFE stream, per block ---------------------
            for t in range(NBLK):
                slot = t % NRING
                # band readback: rows [2,66) of each r section; r=1 lands in
                # partitions [64,128) so its matmuls run in PE row-group 2-3
                # (row tiling) concurrently with r=0 in row-group 0-1.
                bnd = bpool.tile([128, BW], bddt, tag="bnd", name="bnd")
                bps = pstep(bnd)
                for r in range(2):
                    srcr = AP(bndz.ap().tensor,
                              (slot * 2 + r) * BROWS * BW + 2 * BW,
                              [[BW, W], [1, BW]])
                    eng = nc.sync if r == 0 else nc.scalar
                    eng.dma_start(bnd[64 * r:64 * (r + 1), :], srcr)

                # CARAFE: psum[c, (a,jl,b)] += xT_row.T @ band_p
                for ct in range(2):
                    pso = [ps_o_p.tile([128, 256], f32, tag=f"ps_o{r}",
                                       name=f"ps_o{r}") for r in range(2)]
                    for p in range(K5):
                        for r in range(2):
                            rhs = AP(bnd[:].tensor, 64 * r * bps + 2 * p,
                                     [[bps, W], [10, 2], [20, W], [1, 2]])
                            nc.tensor.matmul(
                                pso[r][:],
                                t_xT[2 * t + p][64 * r:64 * (r + 1),
                                                128 * ct:128 * (ct + 1)],
                                rhs, start=(p == 0), stop=(p == K5 - 1))
                    t_o = opool.tile([128, 512], bddt, tag=f"osb{ct}", name=f"osb{ct}")
                    for r in range(2):
                        nc.vector.tensor_copy(t_o[:, 256 * r:256 * (r + 1)], pso[r][:])
                    # DMA to out[c, hr, j]: cols (r,a,jl,b) = 4 contiguous hr rows
                    dsto = AP(out.ap().tensor,
                              ct * 128 * HO * WO + 4 * t * WO,
                              [[HO * WO, 128], [1, 512]])
                    eng = nc.scalar if ct == 0 else nc.sync
                    eng.dma_start(dsto, t_o[:])

    nc.compile()
    return nc


_CACHE = {}


def _get_program():
    if "nc" not in _CACHE:
        _CACHE["nc"] = _build_program()
    return _CACHE["nc"]


def host_prep(x, w_comp, b_comp, w_enc, b_enc):
    """Build per-core input maps."""
    import ml_dtypes
    bf16 = ml_dtypes.bfloat16
    x = np.asarray(x, dtype=np.float32)
    wcT = np.ascontiguousarray(np.asarray(w_comp, np.float32).reshape(CC, C).T)
    bcc = np.asarray(b_comp, np.float32).reshape(CC, 1)
    # weT[tap, cin, cout]
    weT = np.ascontiguousarray(
        np.asarray(w_enc, np.float32).reshape(CM, CC, 9).transpose(2, 1, 0))
    be = np.asarray(b_enc, np.float32).reshape(CM, 1)
    sel = np.zeros((CM, 4), np.float32)
    sel[np.arange(CM), np.arange(CM) % 4] = 1.0
    ident = np.eye(128, dtype=np.float32)

    in_maps = []
    for core in range(NCORES):
        n, half = core // 2, core % 2
        h0 = RH * half
        slab = np.zeros((C, SLAB, W), np.float32)
        r_lo, r_hi = h0 - 2, h0 + SLAB - 2       # x rows [r_lo, r_hi)
        v_lo, v_hi = max(0, r_lo), min(H, r_hi)
        slab[:, v_lo - r_lo:v_hi - r_lo, :] = x[n, :, v_lo:v_hi, :]
        xsT = np.ascontiguousarray(
            slab.reshape(C, SLAB * W).T).astype(bf16)
        in_maps.append({"xs": slab, "xsT": xsT, "wcT": wcT, "bcc": bcc,
                        "weT": weT, "be": be, "sel01": sel, "ident": ident})
    return in_maps


def host_gather(results):
    out = np.empty((N_B, C, S * H, S * W), np.float32)
    for core in range(NCORES):
        n, half = core // 2, core % 2
        out[n, :, HO * half:HO * (half + 1), :] = \
            results[core]["out"].astype(np.float32)
    return out


def kernel(x, w_comp, b_comp, w_enc, b_enc):
    from concourse.bass_utils import run_bass_kernel_spmd
    nc = _get_program()
    in_maps = host_prep(x, w_comp, b_comp, w_enc, b_enc)
    res = run_bass_kernel_spmd(nc, in_maps, list(range(NCORES)))
    return host_gather(res.results)


# revision 36
# speedup vs baseline: 1.2944x; 1.0679x over previous
"""CARAFE (content-aware upsampling) Trainium2 kernel.

Full inputs -> shard over 8 NeuronCores (batch x image-half) -> bass/Tile
kernel per core -> gather full output.

Reference semantics:
  comp = conv1x1(x, w_comp) + b_comp                    [n,64,64,64]
  mask = conv3x3(comp, w_enc, pad=1) + b_enc            [n,100,64,64]
  m    = softmax over 25 of pixel_shuffle(mask, 2)      [n,25,128,128]
  out[n,c,i,j] = sum_k m[n,k,i,j] * xpad[n,c,i//2+p, j//2+q],  k=5p+q

Band construction (v3): per h-pair block and low-res row r, mask values
live in a DRAM "band" image of 68 rows x 1280 cols:
  row = jl + q   (rows 0,1,66,67 are pad; valid w' = row-2)
  col = 20*jl + 10*a + 2*p + b
Nonzero cells of each row form one contiguous 100-el run, so the
SBUF->DRAM scatter is 1 flat shear DMA per (block, r).  Bands are read
back as [64 w', 1280] tiles and contracted against per-row xT tiles on
the PE array.  16-slot ring, zeroed once via the gpsimd software DMA
queue; slot cells are fully overwritten each pass so no re-zeroing is
needed.
"""
import numpy as np
import sys
from contextlib import ExitStack

sys.path.insert(0, "/opt/trn_rl_repo")

# ---------------- problem constants (hardcoded per spec) ----------------
N_B, C, H, W = 4, 256, 64, 64
CC = 64            # compressed channels
K5 = 5             # carafe kernel
S = 2              # scale
CM = K5 * K5 * S * S   # 100 mask channels
NCORES = 8
RH = H // 2        # 32 low-res rows per core
SLAB = RH + 4      # 36 x-rows per core (h0-2 .. h0+33)
NBLK = RH // 2     # 16 h-pair blocks
PIXC = RH * W      # 2048 low-res pixels per core
HO, WO = 2 * RH, 2 * W   # 64 x 128 output shard

NRING = 16         # band ring slots (one per block)
BW = 20 * W        # 1280 band cols: (jl,a,p,b)
BROWS = W + 4      # 68 band rows: jl + q, 2 pad rows each side

_MM_DT = "float32r"   # comp/mask conv matmul dtype
_BD_DT = "bfloat16"   # carafe path: xT, softmax mask, band, output


def _build_program():
    import concourse.bass as bass
    import concourse.tile as tile
    from concourse import bacc, mybir
    from concourse.ap import AP
    AF = mybir.ActivationFunctionType

    def pstep(t):
        return t[:].ap[0][0]

    f32 = mybir.dt.float32
    mmdt = getattr(mybir.dt, _MM_DT)
    bddt = getattr(mybir.dt, _BD_DT)

    nc = bacc.Bacc("TRN2", target_bir_lowering=False, debug=False,
                   num_devices=NCORES)

    # ---------------- DRAM parameters ----------------
    xs = nc.dram_tensor("xs", [C, SLAB, W], mmdt, kind="ExternalInput")
    xsT = nc.dram_tensor("xsT", [SLAB * W, C], bddt, kind="ExternalInput")
    bndz = nc.dram_tensor("bndz", [NRING, 2, BROWS, BW], bddt)
    wcT = nc.dram_tensor("wcT", [C, CC], mmdt, kind="ExternalInput")
    bcc = nc.dram_tensor("bcc", [CC, 1], f32, kind="ExternalInput")
    weT = nc.dram_tensor("weT", [9, CC, CM], mmdt, kind="ExternalInput")
    be = nc.dram_tensor("be", [CM, 1], f32, kind="ExternalInput")
    sel01 = nc.dram_tensor("sel01", [CM, 4], mmdt, kind="ExternalInput")
    ident = nc.dram_tensor("ident", [128, 128], mmdt, kind="ExternalInput")
    out = nc.dram_tensor("out", [C, HO, WO], bddt, kind="ExternalOutput")

    COMP_W = W + 2      # 66: comp cols with 1 zero col each side

    with tile.TileContext(nc) as tc:
        with ExitStack() as ctx:
            cpool = ctx.enter_context(tc.tile_pool(name="const", bufs=1))
            xpool = ctx.enter_context(tc.tile_pool(name="xdata", bufs=1))
            work = ctx.enter_context(tc.tile_pool(name="work", bufs=3))
            opool = ctx.enter_context(tc.tile_pool(name="oevac", bufs=6))
            pers = ctx.enter_context(tc.tile_pool(name="pers", bufs=1))
            bpool = ctx.enter_context(tc.tile_pool(name="bndsb", bufs=5))
            ps_comp_p = ctx.enter_context(tc.tile_pool(name="pscomp", bufs=1, space="PSUM"))
            ps_mask_p = ctx.enter_context(tc.tile_pool(name="psmask", bufs=1, space="PSUM"))
            ps_z_p = ctx.enter_context(tc.tile_pool(name="psz", bufs=1, space="PSUM"))
            ps_t_p = ctx.enter_context(tc.tile_pool(name="pst", bufs=1, space="PSUM"))
            ps_o_p = ctx.enter_context(tc.tile_pool(name="pso", bufs=2, space="PSUM"))

            # ---------------- load constants (scalar queue) ----------------
            t_wc = [cpool.tile([128, CC], mmdt, tag=f"wc{k}", name=f"wc{k}") for k in range(2)]
            for k in range(2):
                nc.scalar.dma_start(t_wc[k][:], wcT.ap()[128 * k:128 * (k + 1), :])
            t_bcc = cpool.tile([CC, 1], f32, tag="bcc", name="bcc")
            nc.scalar.dma_start(t_bcc[:], bcc.ap())
            t_be = cpool.tile([CM, 1], f32, tag="be", name="be")
            nc.sync.dma_start(t_be[:], be.ap())
            t_sel = cpool.tile([CM, 4], mmdt, tag="sel", name="sel")
            nc.sync.dma_start(t_sel[:], sel01.ap())
            t_id = cpool.tile([128, 128], mmdt, tag="id", name="id")
            nc.sync.dma_start(t_id[:], ident.ap())

            # ---------------- load x (scalar queue, in slab-row chunks so
            # the comp conv can start before the whole load finishes) ------
            t_x = [xpool.tile([128, SLAB * W], mmdt, tag=f"x{k}", name=f"x{k}") for k in range(2)]
            cuts = [0, 10 * W, 22 * W, SLAB * W]

            def load_x_chunk(h, eng):
                for k in range(2):
                    eng.dma_start(
                        t_x[k][:, cuts[h]:cuts[h + 1]],
                        xs.ap()[128 * k:128 * (k + 1), :, :]
                        .rearrange("p r w -> p (r w)")[:, cuts[h]:cuts[h + 1]])

            load_x_chunk(0, nc.scalar)
            load_x_chunk(1, nc.scalar)
            t_we = cpool.tile([CC, 9 * CM], mmdt, tag="we", name="we")
            src_we = AP(weT.ap().tensor, 0, [[CM, CC], [CC * CM, 9], [1, CM]])
            nc.gpsimd.dma_start(t_we[:], src_we)
            load_x_chunk(2, nc.gpsimd)

            # xT: 35 overlapping row-pair tiles [128=(2 rows x 64 w), 256 c]
            # bf16.  Row s sits at partitions [0,64) of tile s AND [64,128)
            # of tile s-1, so both PE row-groups can be fed for row tiling.
            # All on the sync queue, which is otherwise idle during startup.
            t_xT = [xpool.tile([128, C], bddt, tag=f"xT{t}", name=f"xT{t}")
                    for t in range(SLAB - 1)]
            for t in range(SLAB - 1):
                nc.sync.dma_start(t_xT[t][:], xsT.ap()[W * t:W * t + 128, :])

            # ------------- zero the band ring once (gpsimd sw queue) -------
            t_zero = pers.tile([BROWS, BW], bddt, tag="zero", name="zero")
            nc.vector.memset(t_zero[:], 0.0)
            for s in range(NRING):
                for r in range(2):
                    nc.gpsimd.dma_start(bndz.ap()[s, r, :, :], t_zero[:])

            # ---------------- comp = 1x1 conv + bias (rows 1..34 of slab) ----
            # comp stored [CC, 34, 66] with zero cols 0 and 65
            t_comp = pers.tile([CC, (RH + 2) * COMP_W], mmdt, tag="comp", name="comp")
            compv = t_comp[:].rearrange("p (r w) -> p r w", w=COMP_W)
            nc.vector.memset(compv[:, :, 0:1].bitcast(f32), 0.0)
            nc.vector.memset(compv[:, :, COMP_W - 1:COMP_W].bitcast(f32), 0.0)

            NPIX_C = (RH + 2) * W  # 2176 pixels (rows 1..34 of slab)
            ctile = 512
            nct = (NPIX_C + ctile - 1) // ctile
            for nt in range(nct):
                p0 = nt * ctile
                n = min(ctile, NPIX_C - p0)
                ps = ps_comp_p.tile([CC, ctile], f32, tag="ps_comp", name="ps_comp")
                for k in range(2):
                    rhs = AP(t_x[k][:].tensor, W + p0, [[pstep(t_x[k]), 128], [1, n]])
                    nc.tensor.matmul(ps[:, :n], t_wc[k][:], rhs,
                                     start=(k == 0), stop=(k == 1))
                # evacuate into comp[., rows, 1:65]; bias added here
                r0 = p0 // W
                nr = n // W
                src_ps = ps[:, :n].rearrange("p (r w) -> p r w", w=W)
                nc.scalar.activation(compv[:, r0:r0 + nr, 1:1 + W], src_ps,
                                     func=AF.Identity, bias=t_bcc[:])

            # ---------------- mask conv 3x3 -> exp -> Z ----------------
            # emask [104, RH*W]: rows 0..99 exp(mask), rows 100..103 Z per ab
            t_em = pers.tile([CM + 4, PIXC], mmdt, tag="emask", name="emask")
            emv = t_em[:].rearrange("p (r w) -> p r w", w=W)
            mtile = 512
            for nt in range(PIXC // mtile):
                mr0 = nt * mtile // W   # 8 mask rows per tile
                ps = ps_mask_p.tile([CM, mtile], f32, tag="ps_mask", name="ps_mask")
                first = True
                for dy in range(3):
                    for dx in range(3):
                        tap = dy * 3 + dx
                        rhs = compv[:, mr0 + dy:mr0 + dy + 8, dx:dx + W]
                        nc.tensor.matmul(ps[:],
                                         t_we[:, tap * CM:(tap + 1) * CM],
                                         rhs, start=first,
                                         stop=(tap == 8))
                        first = False
                # exp(mask + be) -> emask rows 0..99
                dst = emv[0:CM, mr0:mr0 + 8, :]
                nc.scalar.activation(dst, ps[:].rearrange("p (r w) -> p r w", w=W),
                                     func=AF.Exp, bias=t_be[:])
                # Z = sel01.T @ emask chunk -> rows 100..104 (via staging DMA:
                # engine writes must start at partition 0/32/64/96)
                psz = ps_z_p.tile([4, mtile], f32, tag="ps_z", name="ps_z")
                nc.tensor.matmul(psz[:], t_sel[:], t_em[0:CM, nt * mtile:(nt + 1) * mtile],
                                 start=True, stop=True)
                t_zs = work.tile([4, mtile], mmdt, tag="zstage", name="zstage")
                nc.scalar.activation(t_zs[:], psz[:], func=AF.Copy)
                nc.gpsimd.dma_start(emv[CM:CM + 4, mr0:mr0 + 8, :],
                                    t_zs[:].rearrange("p (r w) -> p r w", w=W))

            # ------------- mask side, hoisted for all blocks ---------------
            # transpose -> normalize+permute -> scatter, one band slot per
            # block.  Runs as soon as each mask tile is ready, well before
            # the CARAFE stream needs the band.
            for t in range(NBLK):
                slot = t % NRING
                # transpose emask[:, 2t:2t+2, :] -> [128 pix, 104]
                psT = ps_t_p.tile([128, CM + 4], mmdt, tag="ps_T", name="ps_T")
                src = emv[:, 2 * t:2 * t + 2, :].rearrange("p a b -> p (a b)")
                nc.tensor.transpose(psT[:], src, t_id[0:CM + 4, 0:CM + 4])
                t_emT = work.tile([128, CM + 4], f32, tag="emT", name="emT")
                nc.scalar.activation(t_emT[:], psT[:], func=AF.Copy)

                # reciprocal of Z (4 cols)
                t_rz = work.tile([128, 4], f32, tag="rz", name="rz")
                nc.vector.reciprocal(t_rz[:], t_emT[:, CM:CM + 4])

                # fused softmax-normalize + R'' permutation:
                # t_rp[:, 20q+10a+2p+b] = emT[:, 20p+4q+2a+b] * rz[:, 2a+b]
                t_rp = work.tile([128, CM], bddt, tag="rpp", name="rpp")
                rps = pstep(t_rp)
                emp = pstep(t_emT)
                rzp = pstep(t_rz)
                for a in range(2):
                    dstp = AP(t_rp[:].tensor, 10 * a,
                              [[rps, 128], [20, 5], [2, 5], [1, 2]])
                    in0 = AP(t_emT[:].tensor, 2 * a,
                             [[emp, 128], [4, 5], [20, 5], [1, 2]])
                    in1 = AP(t_rz[:].tensor, 2 * a,
                             [[rzp, 128], [0, 5], [0, 5], [1, 2]])
                    nc.vector.tensor_mul(dstp, in0, in1)

                # band scatter: 1 DMA per r, split across the two HW queues.
                # row = jl+q, col = 20jl+10a+2p+b
                with nc.allow_non_contiguous_dma(reason="banded mask scatter"):
                    for r in range(2):
                        dstb = AP(bndz.ap().tensor,
                                  (slot * 2 + r) * BROWS * BW,
                                  [[BW + 20, W], [BW, K5], [1, 20]])
                        srcb = AP(t_rp[:].tensor, (r * W) * rps,
                                  [[rps, W], [20, K5], [1, 20]])
                        nc.gpsimd.dma_start(dstb, srcb)

            # ---------------- CARAFE stream, per block ---------------------
            for t in range(NBLK):
                slot = t % NRING
                # band readback: rows [2,66) of each r section; r=1 lands in
                # partitions [64,128) so its matmuls run in PE row-group 2-3
                # (row tiling) concurrently with r=0 in row-group 0-1.
                bnd = bpool.tile([128, BW], bddt, tag="bnd", name="bnd")
                bps = pstep(bnd)
                for r in range(2):
                    srcr = AP(bndz.ap().tensor,
                              (slot * 2 + r) * BROWS * BW + 2 * BW,
                              [[BW, W], [1, BW]])
                    eng = nc.sync if r == 0 else nc.scalar
                    eng.dma_start(bnd[64 * r:64 * (r + 1), :], srcr)

                # CARAFE: psum[c, (a,jl,b)] += xT_row.T @ band_p
                for ct in range(2):
                    pso = [ps_o_p.tile([128, 256], f32, tag=f"ps_o{r}",
                                       name=f"ps_o{r}") for r in range(2)]
                    for p in range(K5):
                        for r in range(2):
                            rhs = AP(bnd[:].tensor, 64 * r * bps + 2 * p,
                                     [[bps, W], [10, 2], [20, W], [1, 2]])
                            nc.tensor.matmul(
                                pso[r][:],
                                t_xT[2 * t + p][64 * r:64 * (r + 1),
                                                128 * ct:128 * (ct + 1)],
                                rhs, start=(p == 0), stop=(p == K5 - 1))
                    t_o = opool.tile([128, 512], bddt, tag=f"osb{ct}", name=f"osb{ct}")
                    for r in range(2):
                        nc.vector.tensor_copy(t_o[:, 256 * r:256 * (r + 1)], pso[r][:])
                    # DMA to out[c, hr, j]: cols (r,a,jl,b) = 4 contiguous hr rows
                    dsto = AP(out.ap().tensor,
                              ct * 128 * HO * WO + 4 * t * WO,
                              [[HO * WO, 128], [1, 512]])
                    eng = nc.scalar if ct == 0 else nc.sync
                    eng.dma_start(dsto, t_o[:])

    nc.compile()
    return nc


_CACHE = {}


def _get_program():
    if "nc" not in _CACHE:
        _CACHE["nc"] = _build_program()
    return _CACHE["nc"]


def host_prep(x, w_comp, b_comp, w_enc, b_enc):
    """Build per-core input maps."""
    import ml_dtypes
    bf16 = ml_dtypes.bfloat16
    x = np.asarray(x, dtype=np.float32)
    wcT = np.ascontiguousarray(np.asarray(w_comp, np.float32).reshape(CC, C).T)
    bcc = np.asarray(b_comp, np.float32).reshape(CC, 1)
    # weT[tap, cin, cout]
    weT = np.ascontiguousarray(
        np.asarray(w_enc, np.float32).reshape(CM, CC, 9).transpose(2, 1, 0))
    be = np.asarray(b_enc, np.float32).reshape(CM, 1)
    sel = np.zeros((CM, 4), np.float32)
    sel[np.arange(CM), np.arange(CM) % 4] = 1.0
    ident = np.eye(128, dtype=np.float32)

    in_maps = []
    for core in range(NCORES):
        n, half = core // 2, core % 2
        h0 = RH * half
        slab = np.zeros((C, SLAB, W), np.float32)
        r_lo, r_hi = h0 - 2, h0 + SLAB - 2       # x rows [r_lo, r_hi)
        v_lo, v_hi = max(0, r_lo), min(H, r_hi)
        slab[:, v_lo - r_lo:v_hi - r_lo, :] = x[n, :, v_lo:v_hi, :]
        xsT = np.ascontiguousarray(
            slab.reshape(C, SLAB * W).T).astype(bf16)
        in_maps.append({"xs": slab, "xsT": xsT, "wcT": wcT, "bcc": bcc,
                        "weT": weT, "be": be, "sel01": sel, "ident": ident})
    return in_maps


def host_gather(results):
    out = np.empty((N_B, C, S * H, S * W), np.float32)
    for core in range(NCORES):
        n, half = core // 2, core % 2
        out[n, :, HO * half:HO * (half + 1), :] = \
            results[core]["out"].astype(np.float32)
    return out


def kernel(x, w_comp, b_comp, w_enc, b_enc):
    from concourse.bass_utils import run_bass_kernel_spmd
    nc = _get_program()
    in_maps = host_prep(x, w_comp, b_comp, w_enc, b_enc)
    res = run_bass_kernel_spmd(nc, in_maps, list(range(NCORES)))
    return host_gather(res.results)
